# revision 27
# baseline (speedup 1.0000x reference)
"""Causal self-attention (B=4, S=2048, D=1024, H=16) on 8 Trainium2 NeuronCores.

Sharding: 8 cores = 4 batches x 2 head-groups (8 heads each).
Per core: QKV projections, flash-style causal attention with scores computed
transposed ([k, q] layout), exp on ScalarE (no max subtraction -- scores are
O(1) here), softmax denominator via an appended ones-column in the attn@V
matmul, out-projection against a W_O column slice.  The inter-core
"all-reduce" for the out-projection (row-parallel W_O) is a host-side sum of
the two head-group partials per batch.

All matmuls are bf16 (fp32 PSUM accumulation).  Causal structure is exploited
exactly on the diagonal 128-key chunks: the scores matmul and the attn@V
matmul are restricted to the valid query window [128*jj, 512), the exp of the
second diagonal chunk-pair starts at column 256, and a single shared
[128,128] lower-triangle mask zeroes the intra-chunk upper triangle (the
region below each chunk's window is never read by the restricted attn@V).

The attention inner loop is Scalar-bound (exp ~1us vs ~0.75us of PE work per
chunk pair), so projection matmul groups for the NEXT head-pair block are
interleaved one-per-iteration into the attention stream as PE filler.

Softmax normalization: the two denominator rows are copied (fp16) off PSUM,
broadcast across all 128 partitions with two tiny k=1 PE matmuls (no DRAM
roundtrip), inverted with one full-tile fast reciprocal, and multiplied
directly against the attn@V PSUM rows.

Tail: epoch 3 runs attn(3,3) LAST, carrying outproj(3)'s hp=0..2 partial
matmul groups as PE filler; after the final normalization only 8 small
(matmul + add + DMA) groups remain.
"""

import os
import sys

for _p in ("/opt/trn_rl_repo",):
    if _p not in sys.path and os.path.isdir(_p):
        sys.path.insert(0, _p)

import numpy as np

B, S, D, H, DK = 4, 2048, 1024, 16, 64
N_CORES = 8
EC = 512          # e-dims (= head-dim columns) per core: 8 heads x 64
N_D = D // 128    # 8 contraction chunks for projections
N_SC = S // 128   # 16 key chunks
N_QB = S // 512   # 4 query blocks

_CACHE = {}


def _build():
    import concourse.mybir as mybir
    import concourse.tile as tile
    from concourse import bacc
    from contextlib import ExitStack

    fp32 = mybir.dt.float32
    bf16 = mybir.dt.bfloat16
    fp16 = mybir.dt.float16
    AF = mybir.ActivationFunctionType
    Alu = mybir.AluOpType

    nc = bacc.Bacc(trn_type="TRN2", target_bir_lowering=False, debug=False)

    xt_d = nc.dram_tensor("xt", [D, S], bf16, kind="ExternalInput")
    wq_d = nc.dram_tensor("wqt", [D, EC], bf16, kind="ExternalInput")
    wk_d = nc.dram_tensor("wkt", [D, EC], bf16, kind="ExternalInput")
    wv_d = nc.dram_tensor("wvt", [D, EC], bf16, kind="ExternalInput")
    wo_d = nc.dram_tensor("wot", [EC, D], bf16, kind="ExternalInput")
    yt_d = nc.dram_tensor("yt", [D, S], bf16, kind="ExternalOutput")

    with tile.TileContext(nc) as tc, ExitStack() as ctx:
        # ---- persistent results of phase 1 ------------------------------
        proj_out_pool = ctx.enter_context(tc.tile_pool(name="projout", bufs=1))
        qt_sb = [proj_out_pool.tile([128, S], bf16, name=f"qt{ec}", tag=f"qt{ec}") for ec in range(4)]
        kt_sb = [proj_out_pool.tile([128, S], bf16, name=f"kt{ec}", tag=f"kt{ec}") for ec in range(4)]
        # v_sb[sc]: per head h a 128-col stationary block (all m=128 so the
        # attn@V matmuls avoid the m=65 PE penalty):
        #   even h: [V(64) | ones at col 96] -> psum rows 0..64 out, row 96 = n
        #   odd  h: [ones at col 32 | V(64) at 64:128]
        #           -> psum row 32 = n, rows 64..127 = out
        v_sb = [proj_out_pool.tile([128, 8, 128], bf16, name=f"v{sc}", tag=f"v{sc}") for sc in range(N_SC)]

        # ---- input tiles (all bf16) -------------------------------------
        # DMA dispatch is spread over 4 queues so the first projection
        # group's inputs (x + wq) land as early as possible.
        # DMA priority order matches PE consumption order (sb-major blocks):
        # x cols 0:512, wq, wk, wv, x cols 512:2048, wo — round-robin over
        # the three DMA-capable queues so no single queue's dispatch rate
        # gates arrival (the head is bandwidth-bound at ~360 GB/s).
        xw_pool = ctx.enter_context(tc.tile_pool(name="xw", bufs=1))
        xt_sb = [xw_pool.tile([128, S], bf16, name=f"x{d}", tag=f"x{d}")
                 for d in range(N_D)]
        wq_sb = [xw_pool.tile([128, EC], bf16, name=f"wq{d}", tag=f"wq{d}")
                 for d in range(N_D)]
        wk_sb = [xw_pool.tile([128, EC], bf16, name=f"wk{d}", tag=f"wk{d}")
                 for d in range(N_D)]
        wv_sb = [xw_pool.tile([128, EC], bf16, name=f"wv{d}", tag=f"wv{d}")
                 for d in range(N_D)]
        wo_sb = [xw_pool.tile([128, D], bf16, name=f"wo{cc}", tag=f"wo{cc}")
                 for cc in range(4)]
        _qrr = [nc.sync, nc.scalar, nc.gpsimd]
        _dmas = []
        for d in range(N_D):
            _dmas.append((xt_sb[d][:, 0:512], xt_d.ap()[128 * d:128 * (d + 1), 0:512]))
        for d in range(N_D):
            _dmas.append((wq_sb[d][:], wq_d.ap()[128 * d:128 * (d + 1), :]))
        for d in range(N_D):
            _dmas.append((wk_sb[d][:], wk_d.ap()[128 * d:128 * (d + 1), :]))
        for d in range(N_D):
            _dmas.append((wv_sb[d][:], wv_d.ap()[128 * d:128 * (d + 1), :]))
        for d in range(N_D):
            _dmas.append((xt_sb[d][:, 512:S], xt_d.ap()[128 * d:128 * (d + 1), 512:S]))
        for cc in range(4):
            _dmas.append((wo_sb[cc][:], wo_d.ap()[128 * cc:128 * (cc + 1), :]))
        for i, (dst, src) in enumerate(_dmas):
            _qrr[i % 3].dma_start(dst, src)

        # ---- constant lower-triangle mask (keep where q_local >= k_row) --
        const_pool = ctx.enter_context(tc.tile_pool(name="const", bufs=1))
        tri = const_pool.tile([128, 128], bf16, name="tri", tag="tri")
        nc.gpsimd.memset(tri[:], 1.0)
        nc.gpsimd.affine_select(
            out=tri[:], in_=tri[:], compare_op=Alu.is_ge, fill=0.0,
            base=0, pattern=[[1, 128]], channel_multiplier=-1,
        )
        # one-hot fp16 stationaries for the k=64 denominator-broadcast
        # matmuls (row 96 -> head-A denominator, row 32 -> head-B), plus a
        # persistent zeroed fp16 staging tile so the unused contraction rows
        # of the broadcast matmuls read exact zeros.
        eA = const_pool.tile([128, 64], fp16, name="eA", tag="eA")
        nc.gpsimd.memset(eA[:], 0.0)
        nc.gpsimd.memset(eA[96:97, :], 1.0)
        eB = const_pool.tile([128, 64], fp16, name="eB", tag="eB")
        nc.gpsimd.memset(eB[:], 0.0)
        nc.gpsimd.memset(eB[32:33, :], 1.0)
        sbn_c = const_pool.tile([128, 512], fp16, name="sbnc", tag="sbnc")
        nc.gpsimd.memset(sbn_c[:], 0.0)

        ps_score = ctx.enter_context(tc.tile_pool(name="psscore", bufs=2, space="PSUM"))
        ps_av = ctx.enter_context(tc.tile_pool(name="psav", bufs=1, space="PSUM"))
        ps_proj = ctx.enter_context(tc.tile_pool(name="psproj", bufs=2, space="PSUM"))
        attn_pool = ctx.enter_context(tc.tile_pool(name="attn", bufs=7))
        rb_pool = ctx.enter_context(tc.tile_pool(name="rb", bufs=3))
        outn_pool = ctx.enter_context(tc.tile_pool(name="outn", bufs=4))
        y_pool = ctx.enter_context(tc.tile_pool(name="ysb", bufs=3))

        def qk_proj_group(ec, sb_, which):
            w_sb, out_sb = (wq_sb, qt_sb) if which == "q" else (wk_sb, kt_sb)
            ps = ps_proj.tile([128, 512], fp32, name="pp", tag="pp")
            for d in range(N_D):
                nc.tensor.matmul(
                    ps[:],
                    w_sb[d][:, 128 * ec:128 * (ec + 1)],
                    xt_sb[d][:, 512 * sb_:512 * (sb_ + 1)],
                    start=(d == 0), stop=(d == N_D - 1),
                )
            nc.vector.tensor_copy(out_sb[ec][:, 512 * sb_:512 * (sb_ + 1)], ps[:])

        def emit_v_proj(sc):
            ps = ps_proj.tile([128, 512], fp32, name="pv", tag="pp")
            for d in range(N_D):
                nc.tensor.matmul(
                    ps[:],
                    xt_sb[d][:, 128 * sc:128 * (sc + 1)],
                    wv_sb[d][:],
                    start=(d == 0), stop=(d == N_D - 1),
                )
            vt = v_sb[sc]
            nc.gpsimd.memset(vt[:], 0.0)
            for h in range(8):
                if h % 2 == 0:
                    nc.vector.tensor_copy(vt[:, h, 0:64], ps[:, 64 * h:64 * h + 64])
                    nc.gpsimd.memset(vt[:, h, 96:97], 1.0)
                else:
                    nc.gpsimd.memset(vt[:, h, 32:33], 1.0)
                    nc.vector.tensor_copy(vt[:, h, 64:128], ps[:, 64 * h:64 * h + 64])

        def proj_block_groups(j):
            # sb-major: after block j, every head's qt/kt cols 0:512(j+1)
            # and v chunks 0:4(j+1) exist -> attn(qb=j, hp) unlocked for all
            # hp.  Group order matches DMA arrival (wq, wk, wv).
            gs = [(lambda ec, w: (lambda: qk_proj_group(ec, j, w)))(ec, w)
                  for w in ("q", "k") for ec in range(4)]
            gs += [(lambda sc: (lambda: emit_v_proj(sc)))(sc)
                   for sc in range(4 * j, 4 * j + 4)]
            return gs

        def emit_attn(qb, hp, outn, filler=None, bpool=None, btag="pp"):
            hA, hB = 2 * hp, 2 * hp + 1
            qt, kt = qt_sb[hp], kt_sb[hp]
            nkc = 4 * qb + 4

            def win(kc):
                jj = kc - (nkc - 4)
                return 128 * jj if jj > 0 else 0

            def av_mms(ps_o, h, half, at, kc):
                w0 = win(kc)
                nc.tensor.matmul(
                    ps_o[:, w0:512],
                    v_sb[kc][:, h, :],
                    at[:, half, w0:512],
                    start=(kc == 0), stop=(kc == nkc - 1),
                    skip_group_check=True,
                )

            ps_oA = ps_av.tile([128, 512], fp32, name="poA", tag="poA")
            ps_oB = ps_av.tile([128, 512], fp32, name="poB", tag="poB")
            pend = []
            # chunks are processed in pairs: both chunks' score matmuls
            # (k=64) back-to-back, then both exps, then the lagged attn@V
            # matmuls (k=128) — one k-size transition per phase instead of
            # per chunk (~100ns per transition on TRN2).  attn@V lags ~3
            # chunks so the exp->mask chain never gates PE.
            for base in range(0, nkc, 2):
                kcs = [base] + ([base + 1] if base + 1 < nkc else [])
                pss = []
                for kc in kcs:
                    w0 = win(kc)
                    ps_s = ps_score.tile([128, 2, 512], fp32, name="ps", tag="ps")
                    nc.tensor.matmul(
                        ps_s[:, 0, w0:512],
                        kt[0:64, 128 * kc:128 * (kc + 1)],
                        qt[0:64, 512 * qb + w0:512 * (qb + 1)],
                        start=True, stop=True,
                    )
                    nc.tensor.matmul(
                        ps_s[:, 1, w0:512],
                        kt[64:128, 128 * kc:128 * (kc + 1)],
                        qt[64:128, 512 * qb + w0:512 * (qb + 1)],
                        start=True, stop=True,
                    )
                    pss.append((ps_s, kc))
                for ps_s, kc in pss:
                    w0 = win(kc)
                    at = attn_pool.tile([128, 2, 512], bf16, name="at", tag="at")
                    nc.scalar.activation(at[:, :, w0:512], ps_s[:, :, w0:512], AF.Exp, scale=0.125)
                    if kc >= nkc - 4:
                        nc.vector.tensor_mul(at[:, 0, w0:w0 + 128], at[:, 0, w0:w0 + 128], tri[:])
                        nc.vector.tensor_mul(at[:, 1, w0:w0 + 128], at[:, 1, w0:w0 + 128], tri[:])
                    pend.append((at, kc))
                while len(pend) > 3:
                    p = pend.pop(0)
                    av_mms(ps_oA, hA, 0, p[0], p[1])
                    av_mms(ps_oB, hB, 1, p[0], p[1])
                if filler is not None:
                    g = next(filler, None)
                    if g is not None:
                        g()
                        # outproj-sized groups are half a proj group's PE
                        # work; pull a second one to hold the exp pace
                        if getattr(g, "small", False):
                            g2 = next(filler, None)
                            if g2 is not None:
                                g2()
            for p in pend:
                av_mms(ps_oA, hA, 0, p[0], p[1])
                av_mms(ps_oB, hB, 1, p[0], p[1])

            # normalization: the denominators live in psum rows 96 (head A)
            # and 32 (head B).  Copy those rows into the zeroed fp16 staging
            # tile, broadcast them across all 128 partitions with two k=64
            # one-hot PE matmuls (quadrant-disjoint, no k-transition), take
            # one full-tile fast reciprocal (base-0 only!), then scale the
            # attn@V psum rows directly into outn.
            rbi = rb_pool.tile([128, 512], fp32, name="rbi", tag="rbi")
            nc.vector.tensor_copy(sbn_c[96:97, :], ps_oA[96:97, :])
            nc.vector.tensor_copy(sbn_c[32:33, :], ps_oB[32:33, :])
            # bridge the cast->broadcast latency with a filler group so the
            # PE stream never stalls behind the broadcast matmuls
            if filler is not None:
                g = next(filler, None)
                if g is not None:
                    g()
            ps_b = bpool.tile([128, 512], fp32, name="pb", tag=btag)
            nc.tensor.matmul(
                ps_b[0:64, :], eA[64:128, 0:64], sbn_c[64:128, :],
                start=True, stop=True,
            )
            nc.tensor.matmul(
                ps_b[64:128, :], eB[0:64, 0:64], sbn_c[0:64, :],
                start=True, stop=True, skip_group_check=True,
            )
            nc.vector.reciprocal_approx_fast(out=rbi[:, :], in_=ps_b[:, :])
            nc.vector.tensor_mul(outn[hp][0:64, :], ps_oA[0:64, :], rbi[0:64, :])
            nc.vector.tensor_mul(outn[hp][64:128, :], ps_oB[64:128, :], rbi[64:128, :])

        def outproj_group(qb, outn, dc):
            ps = ps_proj.tile([128, 512], fp32, name="py", tag="pp")
            for hp in range(4):
                nc.tensor.matmul(
                    ps[:],
                    wo_sb[hp][:, 128 * dc:128 * (dc + 1)],
                    outn[hp][:],
                    start=(hp == 0), stop=(hp == 3),
                )
            ysb = y_pool.tile([128, 512], bf16, name="y", tag="y")
            nc.vector.tensor_copy(ysb[:], ps[:])
            nc.sync.dma_start(
                yt_d.ap()[128 * dc:128 * (dc + 1), 512 * qb:512 * (qb + 1)],
                ysb[:])

        def outproj_groups(qb, outn):
            gs = [(lambda dc: (lambda: outproj_group(qb, outn, dc)))(dc)
                  for dc in range(8)]
            for g in gs:
                g.small = True
            return gs

        # ---- interleaved emission ---------------------------------------
        # Epoch j runs attn(j, hp=0..3); its stream carries proj block j+1
        # and (from epoch 1 on) outproj(j-1) as PE filler.  outproj(3) is
        # split: hp=0..2 partials ride attn(3,3), hp=3 finals are the tail.
        outn_all = {qb: [outn_pool.tile([128, 512], bf16, name=f"on{qb}{hp}", tag=f"on{hp}")
                         for hp in range(4)] for qb in range(N_QB)}
        part_pool = ctx.enter_context(tc.tile_pool(name="part", bufs=1))
        part_sb = [part_pool.tile([128, 512], fp32, name=f"pt{dc}", tag=f"pt{dc}")
                   for dc in range(8)]
        noop = lambda: None

        def op3_partial_group(dc):
            # outproj(3) partial over hp=0..2 -> SBUF; hp=3 lands in the tail
            ps = ps_proj.tile([128, 512], fp32, name="pyp", tag="pp")
            for i, hp in enumerate((0, 1, 2)):
                nc.tensor.matmul(
                    ps[:],
                    wo_sb[hp][:, 128 * dc:128 * (dc + 1)],
                    outn_all[3][hp][:],
                    start=(i == 0), stop=(i == 2),
                )
            nc.vector.tensor_copy(part_sb[dc][:], ps[:])

        for g in proj_block_groups(0):
            g()
        for j in range(3):
            gs = proj_block_groups(j + 1)
            if j >= 1:
                gs += outproj_groups(j - 1, outn_all[j - 1])
            f = iter(gs)
            for hp in range(4):
                emit_attn(j, hp, outn_all[j], f, bpool=ps_proj)
            for g in f:
                g()
        f = iter([noop, noop] + outproj_groups(2, outn_all[2]))
        for hp in range(3):
            emit_attn(3, hp, outn_all[3], f, bpool=ps_proj)
        for g in f:
            g()
        f3 = iter([noop] + [(lambda dc: (lambda: op3_partial_group(dc)))(dc)
                            for dc in range(8)])
        emit_attn(3, 3, outn_all[3], f3, bpool=ps_proj)
        for g in f3:
            g()
        # tail: hp=3 finals.  4-deep PSUM rotation (pp x2 + the freed attn@V
        # banks) and adds alternating vector/gpsimd so neither engine
        # serializes the 8 groups.
        for dc in range(8):
            if dc % 4 < 2:
                ps = ps_proj.tile([128, 512], fp32, name="pyf", tag="pp")
            elif dc % 4 == 2:
                ps = ps_av.tile([128, 512], fp32, name="pfA", tag="poA")
            else:
                ps = ps_av.tile([128, 512], fp32, name="pfB", tag="poB")
            nc.tensor.matmul(
                ps[:],
                wo_sb[3][:, 128 * dc:128 * (dc + 1)],
                outn_all[3][3][:],
                start=True, stop=True,
            )
            ysb = y_pool.tile([128, 512], bf16, name="y", tag="y")
            nc.vector.tensor_add(ysb[:], part_sb[dc][:], ps[:])
            nc.sync.dma_start(
                yt_d.ap()[128 * dc:128 * (dc + 1), 512 * 3:512 * 4],
                ysb[:])

    nc.compile()
    return nc


def _get_nc():
    if "nc" not in _CACHE:
        _CACHE["nc"] = _build()
    return _CACHE["nc"]


def _run(in_maps, trace=False, **kw):
    from concourse.bass_utils import run_bass_kernel_spmd
    nc = _get_nc()
    return run_bass_kernel_spmd(nc, in_maps, core_ids=list(range(N_CORES)),
                                trace=trace, **kw)


def _prep_inputs(x, W_Q, W_K, W_V, W_O):
    import ml_dtypes
    bf = ml_dtypes.bfloat16
    x = np.asarray(x, dtype=np.float32)
    W_Q = np.asarray(W_Q, dtype=np.float32)
    W_K = np.asarray(W_K, dtype=np.float32)
    W_V = np.asarray(W_V, dtype=np.float32)
    W_O = np.asarray(W_O, dtype=np.float32)
    in_maps = []
    for c in range(N_CORES):
        b, hg = divmod(c, 2)
        es = EC * hg
        in_maps.append({
            "xt": np.ascontiguousarray(x[b].T).astype(bf),
            "wqt": np.ascontiguousarray(W_Q[es:es + EC, :].T).astype(bf),
            "wkt": np.ascontiguousarray(W_K[es:es + EC, :].T).astype(bf),
            "wvt": np.ascontiguousarray(W_V[es:es + EC, :].T).astype(bf),
            "wot": np.ascontiguousarray(W_O[:, es:es + EC].T).astype(bf),
        })
    return in_maps


def _gather(results):
    y = np.empty((B, S, D), dtype=np.float32)
    for b in range(B):
        yt = results[2 * b]["yt"].astype(np.float32) + results[2 * b + 1]["yt"].astype(np.float32)
        y[b] = yt.T
    return y


def kernel(x, W_Q, W_K, W_V, W_O):
    in_maps = _prep_inputs(x, W_Q, W_K, W_V, W_O)
    res = _run(in_maps, trace=False)
    return _gather(res.results)


# revision 30
# speedup vs baseline: 1.0010x; 1.0010x over previous
"""Causal self-attention (B=4, S=2048, D=1024, H=16) on 8 Trainium2 NeuronCores.

Sharding: 8 cores = 4 batches x 2 head-groups (8 heads each).
Per core: QKV projections, flash-style causal attention with scores computed
transposed ([k, q] layout), exp on ScalarE (no max subtraction -- scores are
O(1) here), softmax denominator via an appended ones-column in the attn@V
matmul, out-projection against a W_O column slice.  The inter-core
"all-reduce" for the out-projection (row-parallel W_O) is a host-side sum of
the two head-group partials per batch.

All matmuls are bf16 (fp32 PSUM accumulation).  Causal structure is exploited
exactly on the diagonal 128-key chunks: the scores matmul and the attn@V
matmul are restricted to the valid query window [128*jj, 512), the exp of the
second diagonal chunk-pair starts at column 256, and a single shared
[128,128] lower-triangle mask zeroes the intra-chunk upper triangle (the
region below each chunk's window is never read by the restricted attn@V).

Emission is sb-major: proj block j (all heads' q/k for query block j + the
next four V chunks) unlocks epoch j = attn(j, hp=0..3); epoch j's attention
stream carries proj block j+1 and outproj(j-1) as PE filler.  Input DMAs are
priority-ordered to match (x cols 0:512, wq, wk, wv, x rest, wo) and
round-robined over the three DMA-capable queues.

Attention chunks are processed in pairs (both chunks' k=64 score matmuls,
then both exps, then the lagged k=128 attn@V matmuls) because k-size
transitions on the PE cost ~100ns each; same-k chains on disjoint row-halves
run quadrant-concurrent.

Softmax normalization: the two denominator rows are copied (fp16) into a
persistent zeroed staging tile, broadcast across all 128 partitions with two
k=64 one-hot PE matmuls (no DRAM roundtrip), inverted with one full-tile
fast reciprocal, and multiplied directly against the attn@V PSUM rows.

Tail: epoch 3 runs attn(3,3) LAST, carrying outproj(3)'s hp=0..2 partial
matmul groups as PE filler; after the final normalization only 8 small
(matmul + add + DMA) groups remain.  y is written bf16 (the host sums the
two head-group partials per batch in fp32).
"""

import os
import sys

for _p in ("/opt/trn_rl_repo",):
    if _p not in sys.path and os.path.isdir(_p):
        sys.path.insert(0, _p)

import numpy as np

B, S, D, H, DK = 4, 2048, 1024, 16, 64
N_CORES = 8
EC = 512          # e-dims (= head-dim columns) per core: 8 heads x 64
N_D = D // 128    # 8 contraction chunks for projections
N_SC = S // 128   # 16 key chunks
N_QB = S // 512   # 4 query blocks

_CACHE = {}


def _build():
    import concourse.mybir as mybir
    import concourse.tile as tile
    from concourse import bacc
    from contextlib import ExitStack

    fp32 = mybir.dt.float32
    bf16 = mybir.dt.bfloat16
    fp16 = mybir.dt.float16
    AF = mybir.ActivationFunctionType
    Alu = mybir.AluOpType

    nc = bacc.Bacc(trn_type="TRN2", target_bir_lowering=False, debug=False)

    xt_d = nc.dram_tensor("xt", [D, S], bf16, kind="ExternalInput")
    wq_d = nc.dram_tensor("wqt", [D, EC], bf16, kind="ExternalInput")
    wk_d = nc.dram_tensor("wkt", [D, EC], bf16, kind="ExternalInput")
    wv_d = nc.dram_tensor("wvt", [D, EC], bf16, kind="ExternalInput")
    wo_d = nc.dram_tensor("wot", [EC, D], bf16, kind="ExternalInput")
    yt_d = nc.dram_tensor("yt", [D, S], bf16, kind="ExternalOutput")

    with tile.TileContext(nc) as tc, ExitStack() as ctx:
        # ---- persistent results of phase 1 ------------------------------
        proj_out_pool = ctx.enter_context(tc.tile_pool(name="projout", bufs=1))
        qt_sb = [proj_out_pool.tile([128, S], bf16, name=f"qt{ec}", tag=f"qt{ec}") for ec in range(4)]
        kt_sb = [proj_out_pool.tile([128, S], bf16, name=f"kt{ec}", tag=f"kt{ec}") for ec in range(4)]
        # v_sb[sc]: per head h a 128-col stationary block (all m=128 so the
        # attn@V matmuls avoid the m=65 PE penalty):
        #   even h: [V(64) | ones at col 96] -> psum rows 0..64 out, row 96 = n
        #   odd  h: [ones at col 32 | V(64) at 64:128]
        #           -> psum row 32 = n, rows 64..127 = out
        v_sb = [proj_out_pool.tile([128, 8, 128], bf16, name=f"v{sc}", tag=f"v{sc}") for sc in range(N_SC)]

        # ---- input tiles (all bf16) -------------------------------------
        # DMA dispatch is spread over 4 queues so the first projection
        # group's inputs (x + wq) land as early as possible.
        # DMA priority order matches PE consumption order (sb-major blocks):
        # x cols 0:512, wq, wk, wv, x cols 512:2048, wo — round-robin over
        # the three DMA-capable queues so no single queue's dispatch rate
        # gates arrival (the head is bandwidth-bound at ~360 GB/s).
        xw_pool = ctx.enter_context(tc.tile_pool(name="xw", bufs=1))
        xt_sb = [xw_pool.tile([128, S], bf16, name=f"x{d}", tag=f"x{d}")
                 for d in range(N_D)]
        wq_sb = [xw_pool.tile([128, EC], bf16, name=f"wq{d}", tag=f"wq{d}")
                 for d in range(N_D)]
        wk_sb = [xw_pool.tile([128, EC], bf16, name=f"wk{d}", tag=f"wk{d}")
                 for d in range(N_D)]
        wv_sb = [xw_pool.tile([128, EC], bf16, name=f"wv{d}", tag=f"wv{d}")
                 for d in range(N_D)]
        wo_sb = [xw_pool.tile([128, D], bf16, name=f"wo{cc}", tag=f"wo{cc}")
                 for cc in range(4)]
        _qrr = [nc.sync, nc.scalar, nc.gpsimd]
        _dmas = []
        for d in range(N_D):
            _dmas.append((xt_sb[d][:, 0:512], xt_d.ap()[128 * d:128 * (d + 1), 0:512]))
        for d in range(N_D):
            _dmas.append((wq_sb[d][:], wq_d.ap()[128 * d:128 * (d + 1), :]))
        for d in range(N_D):
            _dmas.append((wk_sb[d][:], wk_d.ap()[128 * d:128 * (d + 1), :]))
        for d in range(N_D):
            _dmas.append((wv_sb[d][:], wv_d.ap()[128 * d:128 * (d + 1), :]))
        for d in range(N_D):
            _dmas.append((xt_sb[d][:, 512:S], xt_d.ap()[128 * d:128 * (d + 1), 512:S]))
        for cc in range(4):
            _dmas.append((wo_sb[cc][:], wo_d.ap()[128 * cc:128 * (cc + 1), :]))
        for i, (dst, src) in enumerate(_dmas):
            _qrr[i % 3].dma_start(dst, src)

        # ---- constant lower-triangle mask (keep where q_local >= k_row) --
        const_pool = ctx.enter_context(tc.tile_pool(name="const", bufs=1))
        tri = const_pool.tile([128, 128], bf16, name="tri", tag="tri")
        nc.gpsimd.memset(tri[:], 1.0)
        nc.gpsimd.affine_select(
            out=tri[:], in_=tri[:], compare_op=Alu.is_ge, fill=0.0,
            base=0, pattern=[[1, 128]], channel_multiplier=-1,
        )
        # one-hot fp16 stationaries for the k=64 denominator-broadcast
        # matmuls (row 96 -> head-A denominator, row 32 -> head-B), plus a
        # persistent zeroed fp16 staging tile so the unused contraction rows
        # of the broadcast matmuls read exact zeros.
        eA = const_pool.tile([128, 64], fp16, name="eA", tag="eA")
        nc.gpsimd.memset(eA[:], 0.0)
        nc.gpsimd.memset(eA[96:97, :], 1.0)
        eB = const_pool.tile([128, 64], fp16, name="eB", tag="eB")
        nc.gpsimd.memset(eB[:], 0.0)
        nc.gpsimd.memset(eB[32:33, :], 1.0)
        sbn_c = const_pool.tile([128, 512], fp16, name="sbnc", tag="sbnc")
        nc.gpsimd.memset(sbn_c[:], 0.0)

        ps_score = ctx.enter_context(tc.tile_pool(name="psscore", bufs=2, space="PSUM"))
        ps_av = ctx.enter_context(tc.tile_pool(name="psav", bufs=1, space="PSUM"))
        ps_proj = ctx.enter_context(tc.tile_pool(name="psproj", bufs=2, space="PSUM"))
        attn_pool = ctx.enter_context(tc.tile_pool(name="attn", bufs=7))
        rb_pool = ctx.enter_context(tc.tile_pool(name="rb", bufs=3))
        outn_pool = ctx.enter_context(tc.tile_pool(name="outn", bufs=4))
        y_pool = ctx.enter_context(tc.tile_pool(name="ysb", bufs=3))

        def qk_proj_group(ec, sb_, which):
            w_sb, out_sb = (wq_sb, qt_sb) if which == "q" else (wk_sb, kt_sb)
            ps = ps_proj.tile([128, 512], fp32, name="pp", tag="pp")
            for d in range(N_D):
                nc.tensor.matmul(
                    ps[:],
                    w_sb[d][:, 128 * ec:128 * (ec + 1)],
                    xt_sb[d][:, 512 * sb_:512 * (sb_ + 1)],
                    start=(d == 0), stop=(d == N_D - 1),
                )
            nc.vector.tensor_copy(out_sb[ec][:, 512 * sb_:512 * (sb_ + 1)], ps[:])

        def emit_v_proj(sc):
            ps = ps_proj.tile([128, 512], fp32, name="pv", tag="pp")
            for d in range(N_D):
                nc.tensor.matmul(
                    ps[:],
                    xt_sb[d][:, 128 * sc:128 * (sc + 1)],
                    wv_sb[d][:],
                    start=(d == 0), stop=(d == N_D - 1),
                )
            vt = v_sb[sc]
            nc.gpsimd.memset(vt[:], 0.0)
            for h in range(8):
                if h % 2 == 0:
                    nc.vector.tensor_copy(vt[:, h, 0:64], ps[:, 64 * h:64 * h + 64])
                    nc.gpsimd.memset(vt[:, h, 96:97], 1.0)
                else:
                    nc.gpsimd.memset(vt[:, h, 32:33], 1.0)
                    nc.vector.tensor_copy(vt[:, h, 64:128], ps[:, 64 * h:64 * h + 64])

        def proj_block_groups(j):
            # sb-major: after block j, every head's qt/kt cols 0:512(j+1)
            # and v chunks 0:4(j+1) exist -> attn(qb=j, hp) unlocked for all
            # hp.  Group order matches DMA arrival (wq, wk, wv).
            gs = [(lambda ec, w: (lambda: qk_proj_group(ec, j, w)))(ec, w)
                  for w in ("q", "k") for ec in range(4)]
            gs += [(lambda sc: (lambda: emit_v_proj(sc)))(sc)
                   for sc in range(4 * j, 4 * j + 4)]
            return gs

        def emit_attn(qb, hp, outn, filler=None, bpool=None, btag="pp"):
            hA, hB = 2 * hp, 2 * hp + 1
            qt, kt = qt_sb[hp], kt_sb[hp]
            nkc = 4 * qb + 4

            def win(kc):
                jj = kc - (nkc - 4)
                return 128 * jj if jj > 0 else 0

            def av_mms(ps_o, h, half, at, kc):
                w0 = win(kc)
                nc.tensor.matmul(
                    ps_o[:, w0:512],
                    v_sb[kc][:, h, :],
                    at[:, half, w0:512],
                    start=(kc == 0), stop=(kc == nkc - 1),
                    skip_group_check=True,
                )

            ps_oA = ps_av.tile([128, 512], fp32, name="poA", tag="poA")
            ps_oB = ps_av.tile([128, 512], fp32, name="poB", tag="poB")
            pend = []
            # chunks are processed in pairs: both chunks' score matmuls
            # (k=64) back-to-back, then both exps, then the lagged attn@V
            # matmuls (k=128) — one k-size transition per phase instead of
            # per chunk (~100ns per transition on TRN2).  attn@V lags ~3
            # chunks so the exp->mask chain never gates PE.
            for base in range(0, nkc, 2):
                kcs = [base] + ([base + 1] if base + 1 < nkc else [])
                pss = []
                for kc in kcs:
                    w0 = win(kc)
                    ps_s = ps_score.tile([128, 2, 512], fp32, name="ps", tag="ps")
                    nc.tensor.matmul(
                        ps_s[:, 0, w0:512],
                        kt[0:64, 128 * kc:128 * (kc + 1)],
                        qt[0:64, 512 * qb + w0:512 * (qb + 1)],
                        start=True, stop=True,
                    )
                    nc.tensor.matmul(
                        ps_s[:, 1, w0:512],
                        kt[64:128, 128 * kc:128 * (kc + 1)],
                        qt[64:128, 512 * qb + w0:512 * (qb + 1)],
                        start=True, stop=True,
                    )
                    pss.append((ps_s, kc))
                for ps_s, kc in pss:
                    w0 = win(kc)
                    at = attn_pool.tile([128, 2, 512], bf16, name="at", tag="at")
                    nc.scalar.activation(at[:, :, w0:512], ps_s[:, :, w0:512], AF.Exp, scale=0.125)
                    if kc >= nkc - 4:
                        nc.vector.tensor_mul(at[:, 0, w0:w0 + 128], at[:, 0, w0:w0 + 128], tri[:])
                        nc.vector.tensor_mul(at[:, 1, w0:w0 + 128], at[:, 1, w0:w0 + 128], tri[:])
                    pend.append((at, kc))
                while len(pend) > 3:
                    p = pend.pop(0)
                    av_mms(ps_oA, hA, 0, p[0], p[1])
                    av_mms(ps_oB, hB, 1, p[0], p[1])
                if filler is not None:
                    g = next(filler, None)
                    if g is not None:
                        g()
            for p in pend:
                av_mms(ps_oA, hA, 0, p[0], p[1])
                av_mms(ps_oB, hB, 1, p[0], p[1])

            # normalization: the denominators live in psum rows 96 (head A)
            # and 32 (head B).  Copy those rows into the zeroed fp16 staging
            # tile, broadcast them across all 128 partitions with two k=64
            # one-hot PE matmuls (quadrant-disjoint, no k-transition), take
            # one full-tile fast reciprocal (base-0 only!), then scale the
            # attn@V psum rows directly into outn.
            rbi = rb_pool.tile([128, 512], fp32, name="rbi", tag="rbi")
            nc.vector.tensor_copy(sbn_c[96:97, :], ps_oA[96:97, :])
            nc.vector.tensor_copy(sbn_c[32:33, :], ps_oB[32:33, :])
            # bridge the cast->broadcast latency with a filler group so the
            # PE stream never stalls behind the broadcast matmuls
            if filler is not None:
                g = next(filler, None)
                if g is not None:
                    g()
            ps_b = bpool.tile([128, 512], fp32, name="pb", tag=btag)
            nc.tensor.matmul(
                ps_b[0:64, :], eA[64:128, 0:64], sbn_c[64:128, :],
                start=True, stop=True,
            )
            nc.tensor.matmul(
                ps_b[64:128, :], eB[0:64, 0:64], sbn_c[0:64, :],
                start=True, stop=True, skip_group_check=True,
            )
            nc.vector.reciprocal_approx_fast(out=rbi[:, :], in_=ps_b[:, :])
            nc.vector.tensor_mul(outn[hp][0:64, :], ps_oA[0:64, :], rbi[0:64, :])
            nc.vector.tensor_mul(outn[hp][64:128, :], ps_oB[64:128, :], rbi[64:128, :])

        def outproj_group(qb, outn, dc):
            ps = ps_proj.tile([128, 512], fp32, name="py", tag="pp")
            for hp in range(4):
                nc.tensor.matmul(
                    ps[:],
                    wo_sb[hp][:, 128 * dc:128 * (dc + 1)],
                    outn[hp][:],
                    start=(hp == 0), stop=(hp == 3),
                )
            ysb = y_pool.tile([128, 512], bf16, name="y", tag="y")
            nc.vector.tensor_copy(ysb[:], ps[:])
            nc.sync.dma_start(
                yt_d.ap()[128 * dc:128 * (dc + 1), 512 * qb:512 * (qb + 1)],
                ysb[:])

        def outproj_groups(qb, outn):
            return [(lambda dc: (lambda: outproj_group(qb, outn, dc)))(dc)
                    for dc in range(8)]

        # ---- interleaved emission ---------------------------------------
        # Epoch j runs attn(j, hp=0..3); its stream carries proj block j+1
        # and (from epoch 1 on) outproj(j-1) as PE filler.  outproj(3) is
        # split: hp=0..2 partials ride attn(3,3), hp=3 finals are the tail.
        outn_all = {qb: [outn_pool.tile([128, 512], bf16, name=f"on{qb}{hp}", tag=f"on{hp}")
                         for hp in range(4)] for qb in range(N_QB)}
        part_pool = ctx.enter_context(tc.tile_pool(name="part", bufs=1))
        part_sb = [part_pool.tile([128, 512], fp32, name=f"pt{dc}", tag=f"pt{dc}")
                   for dc in range(8)]
        noop = lambda: None

        def op3_partial_group(dc):
            # outproj(3) partial over hp=0..2 -> SBUF; hp=3 lands in the tail
            ps = ps_proj.tile([128, 512], fp32, name="pyp", tag="pp")
            for i, hp in enumerate((0, 1, 2)):
                nc.tensor.matmul(
                    ps[:],
                    wo_sb[hp][:, 128 * dc:128 * (dc + 1)],
                    outn_all[3][hp][:],
                    start=(i == 0), stop=(i == 2),
                )
            nc.vector.tensor_copy(part_sb[dc][:], ps[:])

        for g in proj_block_groups(0):
            g()
        for j in range(3):
            gs = proj_block_groups(j + 1)
            if j >= 1:
                gs += outproj_groups(j - 1, outn_all[j - 1])
            f = iter(gs)
            for hp in range(4):
                emit_attn(j, hp, outn_all[j], f, bpool=ps_proj)
            for g in f:
                g()
        f = iter([noop, noop] + outproj_groups(2, outn_all[2]))
        for hp in range(3):
            emit_attn(3, hp, outn_all[3], f, bpool=ps_proj)
        for g in f:
            g()
        f3 = iter([noop] + [(lambda dc: (lambda: op3_partial_group(dc)))(dc)
                            for dc in range(8)])
        emit_attn(3, 3, outn_all[3], f3, bpool=ps_proj)
        for g in f3:
            g()
        # tail: hp=3 finals.  4-deep PSUM rotation (pp x2 + the freed attn@V
        # banks) and adds alternating vector/gpsimd so neither engine
        # serializes the 8 groups.
        for dc in range(8):
            if dc % 4 < 2:
                ps = ps_proj.tile([128, 512], fp32, name="pyf", tag="pp")
            elif dc % 4 == 2:
                ps = ps_av.tile([128, 512], fp32, name="pfA", tag="poA")
            else:
                ps = ps_av.tile([128, 512], fp32, name="pfB", tag="poB")
            nc.tensor.matmul(
                ps[:],
                wo_sb[3][:, 128 * dc:128 * (dc + 1)],
                outn_all[3][3][:],
                start=True, stop=True,
            )
            ysb = y_pool.tile([128, 512], bf16, name="y", tag="y")
            nc.vector.tensor_add(ysb[:], part_sb[dc][:], ps[:])
            nc.sync.dma_start(
                yt_d.ap()[128 * dc:128 * (dc + 1), 512 * 3:512 * 4],
                ysb[:])

    nc.compile()
    return nc


def _get_nc():
    if "nc" not in _CACHE:
        _CACHE["nc"] = _build()
    return _CACHE["nc"]


def _run(in_maps, trace=False, **kw):
    from concourse.bass_utils import run_bass_kernel_spmd
    nc = _get_nc()
    return run_bass_kernel_spmd(nc, in_maps, core_ids=list(range(N_CORES)),
                                trace=trace, **kw)


def _prep_inputs(x, W_Q, W_K, W_V, W_O):
    import ml_dtypes
    bf = ml_dtypes.bfloat16
    x = np.asarray(x, dtype=np.float32)
    W_Q = np.asarray(W_Q, dtype=np.float32)
    W_K = np.asarray(W_K, dtype=np.float32)
    W_V = np.asarray(W_V, dtype=np.float32)
    W_O = np.asarray(W_O, dtype=np.float32)
    in_maps = []
    for c in range(N_CORES):
        b, hg = divmod(c, 2)
        es = EC * hg
        in_maps.append({
            "xt": np.ascontiguousarray(x[b].T).astype(bf),
            "wqt": np.ascontiguousarray(W_Q[es:es + EC, :].T).astype(bf),
            "wkt": np.ascontiguousarray(W_K[es:es + EC, :].T).astype(bf),
            "wvt": np.ascontiguousarray(W_V[es:es + EC, :].T).astype(bf),
            "wot": np.ascontiguousarray(W_O[:, es:es + EC].T).astype(bf),
        })
    return in_maps


def _gather(results):
    y = np.empty((B, S, D), dtype=np.float32)
    for b in range(B):
        yt = results[2 * b]["yt"].astype(np.float32) + results[2 * b + 1]["yt"].astype(np.float32)
        y[b] = yt.T
    return y


def kernel(x, W_Q, W_K, W_V, W_O):
    in_maps = _prep_inputs(x, W_Q, W_K, W_V, W_O)
    res = _run(in_maps, trace=False)
    return _gather(res.results)
